# revision 35
# baseline (speedup 1.0000x reference)
"""BitLinear (BitNet 1.58-bit ternary) distributed Trainium2 kernel.

Reference semantics:
    scale = max(mean(|w|), 1e-5)
    w_q   = sign(w) * (|w| > scale/3)          # ternary {-1, 0, 1}
    out   = (x @ w_q.T) * scale                # x: [4, 2048, 2048], w: [2048, 2048]

Sharding: data-parallel over tokens (1024 of 8192 per core), weight
replicated; each core computes the scale locally, so there are no
collectives (cross-core sync points absorb the harness' launch skew
and invite power throttling).

Host-side prep: transpose w to [in, out]; pre-cast x to bf16 and
pre-tile it m-major so every x DMA is contiguous 4KB-per-partition
rows; additionally ship an fp16 copy of w^T. The fp16 copy (half the
bytes) is streamed first and abs-sum-reduced to produce the scale —
fp16 rounding is unbiased, so the mean over 4.2M elements matches the
f32 mean to ~2e-7 relative, far below the threshold sensitivity. The
f32 w then streams exactly once, with quantization tracking it at DMA
pace (no SBUF residency, no re-stream, no post-scale burst). The
cross-partition total is summed and broadcast to all 128 partitions
with a single ones-matmul, and a dummy early matmul pre-fetches the PE
instruction stream so the scale-broadcast matmul fires immediately.

Quantization: ternary, computed doubled so it is exact in bf16:
  ACT path:  wq2 = Sign(w + t) + Sign(w - t)            in {-2, 0, 2}
  DVE path:  wq2 = 2*(w > t) - 2*(w < -t)               in {-2, 0, 2}
with t = scale/3; 9 tiles on the ACT path, 6 on the DVE path, and the
final (latest-arriving) tile split column-wise across both engines to
halve its serial tail. The missing 1/2 is folded into the output
scaling (psum * scale/2).

Matmul: bf16 x bf16 -> fp32 PSUM, K=2048 contracted in 16 accumulating
matmuls, N=512 per PSUM bank. The first two m-tiles run k-outer across
8 PSUM banks so the PE overlaps the quant stream; the remaining six
m-tiles run as clean dense passes (~14us each, ~97% of the warm-PE
roofline).
"""

import sys

sys.path.insert(0, "/opt/trn_rl_repo")

import numpy as np

N_CORES = 8
B, S, D = 4, 2048, 2048        # x: [B, S, D]
OUT = 2048                     # out_features
TOK = B * S                    # 8192 tokens
TPC = TOK // N_CORES           # 1024 tokens per core
KT = D // 128                  # 16 K-tiles of 128
MT = TPC // 128                # 8 M-tiles per core
NT = OUT // 512                # 4 N-tiles of 512
N_ELEM = float(D * OUT)        # elements of w
EPS = 1e-5
M_P1 = 2                       # m-tiles in the k-outer first phase


def build_kernel():
    from concourse import bacc, tile, mybir

    f32 = mybir.dt.float32
    bf16 = mybir.dt.bfloat16
    fp16 = mybir.dt.float16
    Alu = mybir.AluOpType
    Act = mybir.ActivationFunctionType
    X = mybir.AxisListType.X

    nc = bacc.Bacc(None, target_bir_lowering=False)
    x_ext = nc.declare_dram_parameter("x", [TPC, D], bf16, isOutput=False)
    w_ext = nc.declare_dram_parameter("weight", [D, OUT], f32, isOutput=False)
    wh_ext = nc.declare_dram_parameter("wh", [D, OUT], fp16, isOutput=False)
    out_ext = nc.declare_dram_parameter("out", [TPC, OUT], f32, isOutput=True)

    with tile.TileContext(nc) as tc:
        with (
            tc.tile_pool(name="persist", bufs=1) as persist,
            tc.tile_pool(name="whf", bufs=4) as whf_pool,
            tc.tile_pool(name="scr", bufs=2) as scr_pool,
            tc.tile_pool(name="wf32", bufs=7) as wf32_pool,
            tc.tile_pool(name="xbuf", bufs=4) as xbuf_pool,
            tc.tile_pool(name="sgn", bufs=4) as sgn_pool,
            tc.tile_pool(name="outp", bufs=1) as out_pool,
            tc.tile_pool(name="psum", bufs=8, space="PSUM") as psum_pool,
        ):
            wq = persist.tile([128, KT, OUT], bf16)      # quantized w^T (doubled)
            ones = persist.tile([128, 128], f32)
            partials = persist.tile([128, KT], f32)
            partials_d = persist.tile([128, KT // 2], f32)
            tot_d = persist.tile([128, 1], f32)
            tot = persist.tile([128, 1], f32)
            scale_sb = persist.tile([128, 1], f32)
            t_pos = persist.tile([128, 1], f32)
            t_neg = persist.tile([128, 1], f32)
            s_half = persist.tile([128, 1], f32)

            nc.vector.memset(ones[:], 1.0)
            # PE warm-up: fetch PE's IRAM block + park the sequencer early so
            # the scale-broadcast matmul fires the moment its input is ready
            warm = psum_pool.tile([128, 512], f32, tag="psum", name="warm")
            nc.tensor.matmul(
                warm[:, 0:1], ones[:], ones[:, 0:1], start=True, stop=True
            )

            def x_dma(m, gated=False):
                xb = xbuf_pool.tile([128, KT, 128], bf16, tag="xbuf", name=f"xb{m}")
                if gated:
                    nc.vector.tensor_copy(xb[0:1, 0:1, 0:1], t_pos[0:1, 0:1])
                nc.sync.dma_start(
                    xb[:],
                    x_ext[m * 128 : (m + 1) * 128, :].rearrange(
                        "p (k c) -> p k c", k=KT
                    ),
                )
                return xb

            # ---- stream 1: fp16 w, |w| row-sums only (reduce split across
            # ACT (Abs + accum_out) and DVE so the fp16 DMA stream stays the
            # pacer) ----
            for k in range(KT):
                wh = whf_pool.tile([128, OUT], fp16, tag="whf", name=f"wh{k}")
                nc.sync.dma_start(wh[:], wh_ext[k * 128 : (k + 1) * 128, :])
                if k % 2 == 0:
                    scr = scr_pool.tile([128, OUT], fp16, tag="scr", name=f"sc{k}")
                    nc.scalar.activation(
                        scr[:], wh[:], Act.Abs,
                        accum_out=partials[:, k : k + 1],
                    )
                else:
                    nc.vector.tensor_reduce(
                        partials_d[:, k // 2 : k // 2 + 1], wh[:], axis=X,
                        op=Alu.add, apply_absolute_value=True,
                    )

            wts = {}

            # ---- scale: sum partials, broadcast via ones-matmul ----
            nc.vector.tensor_reduce(tot_d[:], partials_d[:], axis=X, op=Alu.add)
            nc.vector.tensor_reduce(tot[:], partials[:, 0:KT:2], axis=X, op=Alu.add)
            nc.vector.tensor_tensor(tot[:], tot[:], tot_d[:], Alu.add)
            pbc = psum_pool.tile([128, 512], f32, tag="psum", name="pbc")
            nc.tensor.matmul(pbc[:, 0:1], ones[:], tot[:], start=True, stop=True)
            nc.vector.tensor_scalar(
                scale_sb[:], pbc[:, 0:1], 1.0 / N_ELEM, EPS, Alu.mult, Alu.max
            )
            nc.vector.tensor_scalar(t_pos[:], scale_sb[:], 1.0 / 3.0, None, Alu.mult)
            nc.vector.tensor_scalar(t_neg[:], scale_sb[:], -1.0 / 3.0, None, Alu.mult)
            nc.vector.tensor_scalar(s_half[:], scale_sb[:], 0.5, None, Alu.mult)

            # ---- quantize one K-tile (doubled ternary), hybrid ACT/DVE ----
            def quantize(k, wt):
                if k == KT - 1:
                    # split the final tile across both engines to halve the
                    # serial quant tail after its (late) arrival
                    H = OUT // 2
                    s1 = sgn_pool.tile([128, H], bf16, tag="sgn", name="s1f")
                    s2 = sgn_pool.tile([128, H], bf16, tag="sgn", name="s2f")
                    nc.scalar.activation(s1[:], wt[:, :H], Act.Sign, bias=t_pos[:, 0:1])
                    nc.scalar.activation(s2[:], wt[:, :H], Act.Sign, bias=t_neg[:, 0:1])
                    nc.vector.tensor_tensor(wq[:, k, :H], s1[:], s2[:], Alu.add)
                    neg = sgn_pool.tile([128, H], bf16, tag="sgn", name="negf")
                    nc.vector.tensor_scalar(
                        wq[:, k, H:], wt[:, H:], t_pos[:, 0:1], 2.0, Alu.is_gt, Alu.mult
                    )
                    nc.vector.tensor_scalar(
                        neg[:], wt[:, H:], t_neg[:, 0:1], 2.0, Alu.is_lt, Alu.mult
                    )
                    nc.vector.tensor_tensor(
                        wq[:, k, H:], wq[:, k, H:], neg[:], Alu.subtract
                    )
                elif k % 2 == 0 or k == 9:
                    s1 = sgn_pool.tile([128, OUT], bf16, tag="sgn", name=f"s1_{k}")
                    s2 = sgn_pool.tile([128, OUT], bf16, tag="sgn", name=f"s2_{k}")
                    nc.scalar.activation(s1[:], wt[:], Act.Sign, bias=t_pos[:, 0:1])
                    nc.scalar.activation(s2[:], wt[:], Act.Sign, bias=t_neg[:, 0:1])
                    nc.vector.tensor_tensor(wq[:, k, :], s1[:], s2[:], Alu.add)
                else:
                    neg = sgn_pool.tile([128, OUT], bf16, tag="sgn", name=f"n_{k}")
                    nc.vector.tensor_scalar(
                        wq[:, k, :], wt[:], t_pos[:, 0:1], 2.0, Alu.is_gt, Alu.mult
                    )
                    nc.vector.tensor_scalar(
                        neg[:], wt[:], t_neg[:, 0:1], 2.0, Alu.is_lt, Alu.mult
                    )
                    nc.vector.tensor_tensor(
                        wq[:, k, :], wq[:, k, :], neg[:], Alu.subtract
                    )

            # ---- stream 2: f32 w exactly once, quantized at DMA pace.
            # Tiles k>=2 are gated on the scale via a corner-write of t_pos
            # into the destination (WAW forces the DMA after it), so the f32
            # stream cannot contend with the fp16 stream pre-scale but
            # launches at full bandwidth the moment scale lands. The copies
            # are emitted with a 6-tile lead over quantization so the DMA
            # triggers unblock well ahead of consumption. ----
            def gate_and_dma(k):
                wt = wf32_pool.tile([128, OUT], f32, tag="wf32", name=f"wt{k}")
                nc.vector.tensor_copy(wt[0:1, 0:1], t_pos[0:1, 0:1])
                nc.sync.dma_start(wt[:], w_ext[k * 128 : (k + 1) * 128, :])
                wts[k] = wt

            xbufs = {m: x_dma(m, gated=True) for m in range(M_P1)}
            for k in range(7):
                gate_and_dma(k)
            for k in range(KT):
                quantize(k, wts[k])
                if k + 7 < KT:
                    gate_and_dma(k + 7)

            # rest of x, after all of w (phase-2 m order; DMA is idle by then)
            for m in range(M_P1, MT):
                xbufs[m] = x_dma(m)

            # ---- matmul: out[m,n] = sum_k x[k,m].T @ wq[k,n] ----
            def do_mtile(ms):
                psums = [
                    psum_pool.tile([128, 512], f32, tag="psum", name=f"ps{i}")
                    for i in range(NT * len(ms))
                ]
                for ki, k in enumerate(range(KT)):
                    for mi, m in enumerate(ms):
                        for n in range(NT):
                            nc.tensor.matmul(
                                psums[mi * NT + n][:],
                                xbufs[m][:, k, :],
                                wq[:, k, n * 512 : (n + 1) * 512],
                                start=(ki == 0),
                                stop=(ki == KT - 1),
                            )
                for mi, m in enumerate(ms):
                    ot = out_pool.tile([128, OUT], f32, tag="outp", name=f"ot{m}")
                    for n in range(NT):
                        nc.scalar.activation(
                            ot[:, n * 512 : (n + 1) * 512],
                            psums[mi * NT + n][:],
                            Act.Copy,
                            scale=s_half[:, 0:1],
                        )
                        nc.sync.dma_start(
                            out_ext[m * 128 : (m + 1) * 128, n * 512 : (n + 1) * 512],
                            ot[:, n * 512 : (n + 1) * 512],
                        )

            do_mtile(list(range(M_P1)))
            for m in range(M_P1, MT):
                do_mtile([m])

    nc.finalize()
    return nc


_NC_CACHE = None


def kernel(x, weight):
    global _NC_CACHE
    import ml_dtypes
    from concourse.bass_utils import run_bass_kernel_spmd

    x = np.asarray(x, dtype=np.float32).reshape(TOK, D)
    weight = np.asarray(weight, dtype=np.float32)
    wT = np.ascontiguousarray(weight.T)                      # [in, out] f32
    wh = wT.astype(np.float16)                               # scale-only copy
    in_maps = []
    for i in range(N_CORES):
        shard_t = x[i * TPC : (i + 1) * TPC].T                      # [in, tok]
        tiled = (
            shard_t.reshape(KT, 128, MT, 128)
            .transpose(2, 1, 0, 3)
            .reshape(MT * 128, KT * 128)
        )
        in_maps.append(
            {"x": np.ascontiguousarray(tiled).astype(ml_dtypes.bfloat16),
             "weight": wT,
             "wh": wh}
        )

    if _NC_CACHE is None:
        _NC_CACHE = build_kernel()
    res = run_bass_kernel_spmd(_NC_CACHE, in_maps, core_ids=list(range(N_CORES)))
    outs = [res.results[i]["out"] for i in range(N_CORES)]
    return np.concatenate(outs, axis=0).reshape(B, S, OUT).astype(np.float32)


# revision 36
# speedup vs baseline: 1.0555x; 1.0555x over previous
"""BitLinear (BitNet 1.58-bit ternary) distributed Trainium2 kernel.

Reference semantics:
    scale = max(mean(|w|), 1e-5)
    w_q   = sign(w) * (|w| > scale/3)          # ternary {-1, 0, 1}
    out   = (x @ w_q.T) * scale                # x: [4, 2048, 2048], w: [2048, 2048]

Sharding: data-parallel over tokens (1024 of 8192 per core), weight
replicated; each core computes the scale locally, so there are no
collectives (cross-core sync points absorb the harness' launch skew
and invite power throttling).

Host-side prep: transpose w to [in, out]; pre-cast x to bf16 and
pre-tile it m-major so every x DMA is contiguous 4KB-per-partition
rows; additionally ship an fp16 copy of w^T. The fp16 copy (half the
bytes) is streamed first and abs-sum-reduced to produce the scale —
fp16 rounding is unbiased, so the mean over 4.2M elements matches the
f32 mean to ~2e-7 relative, far below the threshold sensitivity. The
f32 w then streams exactly once, with quantization tracking it at DMA
pace (no SBUF residency, no re-stream, no post-scale burst). The
cross-partition total is summed and broadcast to all 128 partitions
with a single ones-matmul, and a dummy early matmul pre-fetches the PE
instruction stream so the scale-broadcast matmul fires immediately.

Quantization: ternary, computed doubled so it is exact in bf16:
  ACT path:  wq2 = Sign(w + t) + Sign(w - t)            in {-2, 0, 2}
  DVE path:  wq2 = 2*(w > t) - 2*(w < -t)               in {-2, 0, 2}
with t = scale/3; 9 tiles on the ACT path, 6 on the DVE path, and the
final (latest-arriving) tile split column-wise across both engines to
halve its serial tail. The missing 1/2 is folded into the output
scaling (psum * scale/2).

Matmul: bf16 x bf16 -> fp32 PSUM, K=2048 contracted in 16 accumulating
matmuls, N=512 per PSUM bank. The first two m-tiles run k-outer across
8 PSUM banks so the PE overlaps the quant stream; the remaining six
m-tiles run as clean dense passes (~14us each, ~97% of the warm-PE
roofline).
"""

import sys

sys.path.insert(0, "/opt/trn_rl_repo")

import numpy as np

N_CORES = 8
B, S, D = 4, 2048, 2048        # x: [B, S, D]
OUT = 2048                     # out_features
TOK = B * S                    # 8192 tokens
TPC = TOK // N_CORES           # 1024 tokens per core
KT = D // 128                  # 16 K-tiles of 128
MT = TPC // 128                # 8 M-tiles per core
NT = OUT // 512                # 4 N-tiles of 512
N_ELEM = float(D * OUT)        # elements of w
EPS = 1e-5
M_P1 = 2                       # m-tiles in the k-outer first phase


def build_kernel():
    from concourse import bacc, tile, mybir

    f32 = mybir.dt.float32
    bf16 = mybir.dt.bfloat16
    fp16 = mybir.dt.float16
    Alu = mybir.AluOpType
    Act = mybir.ActivationFunctionType
    X = mybir.AxisListType.X

    nc = bacc.Bacc(None, target_bir_lowering=False)
    x_ext = nc.declare_dram_parameter("x", [TPC, D], bf16, isOutput=False)
    w_ext = nc.declare_dram_parameter("weight", [D, OUT], f32, isOutput=False)
    wh_ext = nc.declare_dram_parameter("wh", [D, OUT], fp16, isOutput=False)
    out_ext = nc.declare_dram_parameter("out", [TPC, OUT], f32, isOutput=True)

    with tile.TileContext(nc) as tc:
        with (
            tc.tile_pool(name="persist", bufs=1) as persist,
            tc.tile_pool(name="whf", bufs=3) as whf_pool,
            tc.tile_pool(name="wf32", bufs=7) as wf32_pool,
            tc.tile_pool(name="xbuf", bufs=4) as xbuf_pool,
            tc.tile_pool(name="sgn", bufs=4) as sgn_pool,
            tc.tile_pool(name="outp", bufs=1) as out_pool,
            tc.tile_pool(name="psum", bufs=8, space="PSUM") as psum_pool,
        ):
            wq = persist.tile([128, KT, OUT], bf16)      # quantized w^T (doubled)
            ones = persist.tile([128, 128], f32)
            partials = persist.tile([128, KT // 4], f32)
            partials_d = persist.tile([128, KT // 4], f32)
            tot_d = persist.tile([128, 1], f32)
            tot = persist.tile([128, 1], f32)
            scale_sb = persist.tile([128, 1], f32)
            t_pos = persist.tile([128, 1], f32)
            t_neg = persist.tile([128, 1], f32)
            s_half = persist.tile([128, 1], f32)

            nc.vector.memset(ones[:], 1.0)
            # PE warm-up: fetch PE's IRAM block + park the sequencer early so
            # the scale-broadcast matmul fires the moment its input is ready
            warm = psum_pool.tile([128, 512], f32, tag="psum", name="warm")
            nc.tensor.matmul(
                warm[:, 0:1], ones[:], ones[:, 0:1], start=True, stop=True
            )

            def x_dma(m):
                xb = xbuf_pool.tile([128, KT, 128], bf16, tag="xbuf", name=f"xb{m}")
                nc.sync.dma_start(
                    xb[:],
                    x_ext[m * 128 : (m + 1) * 128, :].rearrange(
                        "p (k c) -> p k c", k=KT
                    ),
                )
                return xb

            # ---- stream 1: fp16 w in 1-MiB pair transfers (half-MiB DMAs
            # underfill the queues), |w| sums per pair alternating between
            # ACT (in-place Abs + accum_out) and DVE (reduce XY) ----
            for j in range(KT // 2):
                wh = whf_pool.tile([128, 2, OUT], fp16, tag="whf", name=f"wh{j}")
                nc.sync.dma_start(
                    wh[:],
                    wh_ext[j * 256 : (j + 1) * 256, :].rearrange(
                        "(t p) o -> p t o", p=128
                    ),
                )
                if j % 2 == 0:
                    nc.scalar.activation(
                        wh[:], wh[:], Act.Abs,
                        accum_out=partials[:, j // 2 : j // 2 + 1],
                    )
                else:
                    nc.vector.tensor_reduce(
                        partials_d[:, j // 2 : j // 2 + 1], wh[:],
                        axis=mybir.AxisListType.XY,
                        op=Alu.add, apply_absolute_value=True,
                    )

            # x slice for matmul phase 1 (queued right behind the fp16 stream)
            xbufs = {m: x_dma(m) for m in range(M_P1)}

            # first two f32 w tiles prefetch ungated (pipeline warmth)
            wts = {}
            for k in range(2):
                wt = wf32_pool.tile([128, OUT], f32, tag="wf32", name=f"wt{k}")
                nc.sync.dma_start(wt[:], w_ext[k * 128 : (k + 1) * 128, :])
                wts[k] = wt

            # ---- scale: sum partials, broadcast via ones-matmul ----
            nc.vector.tensor_reduce(tot_d[:], partials_d[:], axis=X, op=Alu.add)
            nc.vector.tensor_reduce(tot[:], partials[:], axis=X, op=Alu.add)
            nc.vector.tensor_tensor(tot[:], tot[:], tot_d[:], Alu.add)
            pbc = psum_pool.tile([128, 512], f32, tag="psum", name="pbc")
            nc.tensor.matmul(pbc[:, 0:1], ones[:], tot[:], start=True, stop=True)
            nc.vector.tensor_scalar(
                scale_sb[:], pbc[:, 0:1], 1.0 / N_ELEM, EPS, Alu.mult, Alu.max
            )
            nc.vector.tensor_scalar(t_pos[:], scale_sb[:], 1.0 / 3.0, None, Alu.mult)
            nc.vector.tensor_scalar(t_neg[:], scale_sb[:], -1.0 / 3.0, None, Alu.mult)
            nc.vector.tensor_scalar(s_half[:], scale_sb[:], 0.5, None, Alu.mult)

            # ---- quantize one K-tile (doubled ternary), hybrid ACT/DVE ----
            def quantize(k, wt):
                if k == KT - 1:
                    # split the final tile across both engines to halve the
                    # serial quant tail after its (late) arrival
                    H = OUT // 2
                    s1 = sgn_pool.tile([128, H], bf16, tag="sgn", name="s1f")
                    s2 = sgn_pool.tile([128, H], bf16, tag="sgn", name="s2f")
                    nc.scalar.activation(s1[:], wt[:, :H], Act.Sign, bias=t_pos[:, 0:1])
                    nc.scalar.activation(s2[:], wt[:, :H], Act.Sign, bias=t_neg[:, 0:1])
                    nc.vector.tensor_tensor(wq[:, k, :H], s1[:], s2[:], Alu.add)
                    neg = sgn_pool.tile([128, H], bf16, tag="sgn", name="negf")
                    nc.vector.tensor_scalar(
                        wq[:, k, H:], wt[:, H:], t_pos[:, 0:1], 2.0, Alu.is_gt, Alu.mult
                    )
                    nc.vector.tensor_scalar(
                        neg[:], wt[:, H:], t_neg[:, 0:1], 2.0, Alu.is_lt, Alu.mult
                    )
                    nc.vector.tensor_tensor(
                        wq[:, k, H:], wq[:, k, H:], neg[:], Alu.subtract
                    )
                elif k % 2 == 0 or k == 9:
                    s1 = sgn_pool.tile([128, OUT], bf16, tag="sgn", name=f"s1_{k}")
                    s2 = sgn_pool.tile([128, OUT], bf16, tag="sgn", name=f"s2_{k}")
                    nc.scalar.activation(s1[:], wt[:], Act.Sign, bias=t_pos[:, 0:1])
                    nc.scalar.activation(s2[:], wt[:], Act.Sign, bias=t_neg[:, 0:1])
                    nc.vector.tensor_tensor(wq[:, k, :], s1[:], s2[:], Alu.add)
                else:
                    neg = sgn_pool.tile([128, OUT], bf16, tag="sgn", name=f"n_{k}")
                    nc.vector.tensor_scalar(
                        wq[:, k, :], wt[:], t_pos[:, 0:1], 2.0, Alu.is_gt, Alu.mult
                    )
                    nc.vector.tensor_scalar(
                        neg[:], wt[:], t_neg[:, 0:1], 2.0, Alu.is_lt, Alu.mult
                    )
                    nc.vector.tensor_tensor(
                        wq[:, k, :], wq[:, k, :], neg[:], Alu.subtract
                    )

            # ---- stream 2: f32 w exactly once, quantized at DMA pace.
            # Tiles k>=2 are gated on the scale via a corner-write of t_pos
            # into the destination (WAW forces the DMA after it), so the f32
            # stream cannot contend with the fp16 stream pre-scale but
            # launches at full bandwidth the moment scale lands. The copies
            # are emitted with a 6-tile lead over quantization so the DMA
            # triggers unblock well ahead of consumption. ----
            def gate_and_dma(k):
                wt = wf32_pool.tile([128, OUT], f32, tag="wf32", name=f"wt{k}")
                nc.vector.tensor_copy(wt[0:1, 0:1], t_pos[0:1, 0:1])
                nc.sync.dma_start(wt[:], w_ext[k * 128 : (k + 1) * 128, :])
                wts[k] = wt

            for k in range(2, 7):
                gate_and_dma(k)
            for k in range(KT):
                quantize(k, wts[k])
                if k + 7 < KT:
                    gate_and_dma(k + 7)

            # rest of x, after all of w (phase-2 m order; DMA is idle by then)
            for m in range(M_P1, MT):
                xbufs[m] = x_dma(m)

            # ---- matmul: out[m,n] = sum_k x[k,m].T @ wq[k,n] ----
            def do_mtile(ms):
                psums = [
                    psum_pool.tile([128, 512], f32, tag="psum", name=f"ps{i}")
                    for i in range(NT * len(ms))
                ]
                for ki, k in enumerate(range(KT)):
                    for mi, m in enumerate(ms):
                        for n in range(NT):
                            nc.tensor.matmul(
                                psums[mi * NT + n][:],
                                xbufs[m][:, k, :],
                                wq[:, k, n * 512 : (n + 1) * 512],
                                start=(ki == 0),
                                stop=(ki == KT - 1),
                            )
                for mi, m in enumerate(ms):
                    ot = out_pool.tile([128, OUT], f32, tag="outp", name=f"ot{m}")
                    for n in range(NT):
                        nc.scalar.activation(
                            ot[:, n * 512 : (n + 1) * 512],
                            psums[mi * NT + n][:],
                            Act.Copy,
                            scale=s_half[:, 0:1],
                        )
                        nc.sync.dma_start(
                            out_ext[m * 128 : (m + 1) * 128, n * 512 : (n + 1) * 512],
                            ot[:, n * 512 : (n + 1) * 512],
                        )

            do_mtile(list(range(M_P1)))
            for m in range(M_P1, MT):
                do_mtile([m])

    nc.finalize()
    return nc


_NC_CACHE = None


def kernel(x, weight):
    global _NC_CACHE
    import ml_dtypes
    from concourse.bass_utils import run_bass_kernel_spmd

    x = np.asarray(x, dtype=np.float32).reshape(TOK, D)
    weight = np.asarray(weight, dtype=np.float32)
    wT = np.ascontiguousarray(weight.T)                      # [in, out] f32
    wh = wT.astype(np.float16)                               # scale-only copy
    in_maps = []
    for i in range(N_CORES):
        shard_t = x[i * TPC : (i + 1) * TPC].T                      # [in, tok]
        tiled = (
            shard_t.reshape(KT, 128, MT, 128)
            .transpose(2, 1, 0, 3)
            .reshape(MT * 128, KT * 128)
        )
        in_maps.append(
            {"x": np.ascontiguousarray(tiled).astype(ml_dtypes.bfloat16),
             "weight": wT,
             "wh": wh}
        )

    if _NC_CACHE is None:
        _NC_CACHE = build_kernel()
    res = run_bass_kernel_spmd(_NC_CACHE, in_maps, core_ids=list(range(N_CORES)))
    outs = [res.results[i]["out"] for i in range(N_CORES)]
    return np.concatenate(outs, axis=0).reshape(B, S, OUT).astype(np.float32)
